# revision 1
# baseline (speedup 1.0000x reference)
"""Multi-head attention (B=2, S=2048, D=1024, H=16) on 8 TRN2 NeuronCores.

Sharding: core c handles batch c//4 and heads 4*(c%4) .. 4*(c%4)+4
(tensor-parallel over heads x data-parallel over batch).

Per-core pipeline (bf16 matmuls, fp32 PSUM):
  1. qT/kT = W @ X^T  [d=256 on partitions, s free]; v = X @ W^T [s, d] with
     a ones column appended per head (softmax denominator rides the A*V).
  2. Global 128-step software pipeline over 8 (qs, pair) blocks x 16 k-blocks:
       PE:  S^T[k,q] = kT.T @ qT  (row-packed head pair)       -> sc PSUM
       ACT: p = exp(S/8)                                       -> pt SBUF
       PE:  att[q, d|den] += pt.T @ [v|1]   (pt stationary, v moving: the
            moving stream is 65 columns instead of 512)
     Block end: per-q reciprocal of the den column, broadcast-multiply
     (DVE), then 4 DMA-xbar transposes att[q,d] -> outt[d,q] (no PE/DVE);
     the last block transposes on the then-idle PE (is_transpose+identity).
  3. partial[s, 1024] = outt.T @ WoT, interleaved into the PE slack of the
     last blocks + a small tail; one merged [128, 1024] DMA per s-block.
Host: full output[b] = sum of the 4 partials for batch b + b_o.

Projection/O-proj chains are split into ~430ns pieces and drained from a
deadline-ordered queue into each pipeline step, alternating between two
single-bank PSUM tags so a chain never waits on the previous chain's DVE
evacuation.
"""
from collections import deque

import numpy as np
import ml_dtypes

import concourse.bass as bass
import concourse.mybir as mybir
from concourse.tile import TileContext
from concourse.bass_utils import run_bass_kernel_spmd


def split_multi_waits(nc):
    """This container's walrus codegen allows only one sync-wait command per
    instruction ("Too many sync wait commands" in setupSyncWait). Tile
    sometimes attaches several semaphore waits to one instruction; hoist the
    extras onto dedicated EventSemaphore instructions inserted immediately
    before, on the same engine (sequencers execute in order, so semantics
    are identical)."""
    n = [0]
    for f in nc.m.functions:
        for blk in f.blocks:
            new_insts = []
            changed = False
            for inst in blk.instructions:
                si = inst.sync_info
                if si is not None and len(si.on_wait) > 1:
                    waits = list(si.on_wait)
                    for w in waits[:-1]:
                        n[0] += 1
                        ev = mybir.InstEventSemaphore(
                            name=f"WSPLIT-{n[0]}",
                            ins=[], outs=[],
                        )
                        ev.engine = inst.engine
                        ev.sync_info = mybir.SyncInfo(on_wait=[w], on_update=[])
                        new_insts.append(ev)
                        nc.register_instruction(ev, overwrite=True)
                    inst.sync_info = mybir.SyncInfo(
                        on_wait=[waits[-1]], on_update=list(si.on_update)
                    )
                    changed = True
                new_insts.append(inst)
            if changed:
                blk.instructions = new_insts
    return n[0]


BF16 = mybir.dt.bfloat16
F32 = mybir.dt.float32

B, S, D = 2, 2048, 1024
H, DK = 16, 64
HPC = 4              # heads per core
DC = HPC * DK        # 256 d-model dims per core
N_CORES = 8
P = 128              # partitions
FC = D // P          # 8 feature chunks (contraction for projections)
KB = S // P          # 16 k-blocks
QSUP = 512           # q tile width per block
NQ = S // QSUP       # 4 q supertiles
SSN = S // 512       # 4 s-chunks for projections
NBLK = 2 * NQ        # 8 (qs, pair) blocks
NSTEP = NBLK * KB    # 128 pipeline steps
LAG = 10             # av trails exp by LAG steps (relaxes v-proj deadlines
                     # against the serial DMA feed)


def build_bass():
    nc = bass.Bass()
    xtq = nc.dram_tensor("xtq", [D, S], BF16, kind="ExternalInput")
    xtk = nc.dram_tensor("xtk", [D, S], BF16, kind="ExternalInput")
    xtv = nc.dram_tensor("xtv", [D, S], BF16, kind="ExternalInput")
    wqt = nc.dram_tensor("wqt", [D, DC], BF16, kind="ExternalInput")
    wkt = nc.dram_tensor("wkt", [D, DC], BF16, kind="ExternalInput")
    wvt = nc.dram_tensor("wvt", [D, DC], BF16, kind="ExternalInput")
    wot = nc.dram_tensor("wot", [DC, D], BF16, kind="ExternalInput")
    bq = nc.dram_tensor("bq", [DC, 1], F32, kind="ExternalInput")
    bk = nc.dram_tensor("bk", [DC, 1], F32, kind="ExternalInput")
    bvr = nc.dram_tensor("bvr", [1, DC], F32, kind="ExternalInput")
    ident = nc.dram_tensor("ident", [P, P], BF16, kind="ExternalInput")
    outp = nc.dram_tensor("outp", [S, D], BF16, kind="ExternalOutput")

    # block order: all pair-0 blocks first, so the d-chunk-1 projections and
    # the O-projections get late deadlines
    blocks = [(qs, 0) for qs in range(NQ)] + [(qs, 1) for qs in range(NQ)]

    with TileContext(nc) as tc:
        consts = tc.alloc_tile_pool(name="consts", bufs=1)
        qkv = tc.alloc_tile_pool(name="qkv", bufs=1)
        ptpool = tc.alloc_tile_pool(name="ptpool", bufs=LAG + 2)
        attpool = tc.alloc_tile_pool(name="attpool", bufs=3)
        dyn = tc.alloc_tile_pool(name="dyn", bufs=2)
        opool = tc.alloc_tile_pool(name="opool", bufs=4)
        # PSUM banks: sc 2x2 + av 2x1 + proj ps0/ps1 1x1 each = 8
        sc_pool = tc.alloc_tile_pool(name="sc_pool", bufs=2, space="PSUM")
        av_pool = tc.alloc_tile_pool(name="av_pool", bufs=2, space="PSUM")
        psp = tc.alloc_tile_pool(name="psp", bufs=1, space="PSUM")

        # ---- all input DMAs up front (SP issues back-to-back, no waits;
        # transfers spread across queues; deadline-ordered) ----
        wk_sb = consts.tile([P, FC, DC], BF16, tag="wk")
        wq_sb = consts.tile([P, FC, DC], BF16, tag="wq")
        wv_sb = consts.tile([P, FC, DC], BF16, tag="wv")
        wo_sb = consts.tile([P, 2, D], BF16, tag="wo")
        bq_sb = consts.tile([P, 2, 1], F32, tag="bq")
        bk_sb = consts.tile([P, 2, 1], F32, tag="bk")
        bv_row = consts.tile([1, DC], F32, tag="bvrow")
        xk_sb = consts.tile([P, FC, S], BF16, tag="xk")
        xq_sb = consts.tile([P, FC, S], BF16, tag="xq")
        xv_sb = consts.tile([P, FC, S], BF16, tag="xv")

        xk_r = xtk.rearrange("(c p) s -> p c s", p=P)
        xq_r = xtq.rearrange("(c p) s -> p c s", p=P)
        xv_r = xtv.rearrange("(c p) s -> p c s", p=P)

        # all DMA transfers serialize on the (exclusive) DMA-engine pool at
        # ~360 GB/s; track analytic landing times so compute pieces are not
        # emitted (and head-of-line block the PE) before their data exists
        dma_land = {}            # tensor tag -> estimated landing time (ns)
        dma_state = {"issue": 1000.0, "busy": 1000.0}

        def track_dma(key, nbytes):
            st = dma_state
            st["busy"] = max(st["busy"], st["issue"] + 650.0) + nbytes / 360.0
            st["issue"] += 650.0
            dma_land[key] = st["busy"] + 1550.0

        def x_dma(sb, src, ss, key):
            # two 4KB/partition transfers per s-chunk
            sl = slice(ss * 512, (ss + 1) * 512)
            nc.sync.dma_start(sb[:, 0:4, sl], src[:, 0:4, sl])
            nc.sync.dma_start(sb[:, 4:8, sl], src[:, 4:8, sl])
            track_dma(key, P * 4 * 512 * 2)
            track_dma(key, P * 4 * 512 * 2)

        nc.sync.dma_start(wq_sb[:], wqt.rearrange("(c p) d -> p c d", p=P))
        track_dma("wq", D * DC * 2)
        x_dma(xq_sb, xq_r, 0, "xq0")
        nc.sync.dma_start(wk_sb[:], wkt.rearrange("(c p) d -> p c d", p=P))
        track_dma("wk", D * DC * 2)
        # xk ss0 split by s-columns: kcol 0/1 start on the first half while
        # the second is still in flight
        nc.sync.dma_start(xk_sb[:, :, 0:256], xk_r[:, :, 0:256])
        track_dma("xk0h", P * 8 * 256 * 2)
        nc.sync.dma_start(xk_sb[:, :, 256:512], xk_r[:, :, 256:512])
        track_dma("xk0", P * 8 * 256 * 2)
        nc.sync.dma_start(bk_sb[:], bk.rearrange("(c p) o -> p c o", p=P))
        nc.sync.dma_start(bq_sb[:], bq.rearrange("(c p) o -> p c o", p=P))
        nc.sync.dma_start(bv_row[:], bvr[:])
        track_dma("b", 3 * 1024)
        x_dma(xk_sb, xk_r, 1, "xk1")
        x_dma(xk_sb, xk_r, 2, "xk2")
        nc.sync.dma_start(wv_sb[:], wvt.rearrange("(c p) d -> p c d", p=P))
        track_dma("wv", D * DC * 2)
        x_dma(xv_sb, xv_r, 0, "xv0")
        x_dma(xk_sb, xk_r, 3, "xk3")
        x_dma(xq_sb, xq_r, 1, "xq1")
        x_dma(xv_sb, xv_r, 1, "xv1")
        x_dma(xv_sb, xv_r, 2, "xv2")
        x_dma(xv_sb, xv_r, 3, "xv3")
        x_dma(xq_sb, xq_r, 2, "xq2")
        x_dma(xq_sb, xq_r, 3, "xq3")
        nc.sync.dma_start(wo_sb[:], wot.rearrange("(c p) n -> p c n", p=P))
        track_dma("wo", DC * D * 2)
        ident_sb = consts.tile([P, P], BF16, tag="ident")
        nc.sync.dma_start(ident_sb[:], ident[:])

        T_EXP0 = 10000.0         # first-exp estimate for step-time mapping

        def land_step(*keys):
            t = max(dma_land[k] for k in keys)
            return max(0, int((t - T_EXP0) / 1040.0) + 1)

        # ---- persistent activations ----
        kt_sb = qkv.tile([P, 2, S], BF16, tag="kt")   # [hh*64+d, pair, s]
        qt_sb = qkv.tile([P, 2, S], BF16, tag="qt")
        v_sb = qkv.tile([P, KB, HPC, DK + 1], BF16, tag="v")
        nc.vector.memset(v_sb[:, :, :, DK:], 1.0)
        outt_sb = qkv.tile([P, 2, S], BF16, tag="outt")  # [(hh,d), pair, q]
        ones_f32 = consts.tile([1, P], F32, tag="ones_f32")
        nc.vector.memset(ones_f32[:], 1.0)
        bv_rep = consts.tile([P, DC], F32, tag="bvrep")

        # ---- piece-split projection chains on two alternating psum tags ----
        chain_ctr = [0]

        def next_tag():
            chain_ctr[0] += 1
            return f"ps{chain_ctr[0] % 2}"

        def qk_chain_pieces(x_sb, w_sb, b_sb, t_sb, dc, ss, pool=None, tag=None):
            """4 pieces x 2 matmuls (~430ns each) + evac on the last."""
            tag = tag or next_tag()
            pl = pool or psp
            state = {}

            def piece(i):
                if i == 0:
                    state["ps"] = pl.tile([P, 512], F32, tag=tag, name="ps")
                ps = state["ps"]
                for fc in (2 * i, 2 * i + 1):
                    nc.tensor.matmul(
                        ps[:],
                        w_sb[:, fc, dc * P:(dc + 1) * P],
                        x_sb[:, fc, ss * 512:(ss + 1) * 512],
                        start=(fc == 0), stop=(fc == FC - 1),
                    )
                if i == 3:
                    nc.vector.tensor_scalar(
                        t_sb[:, dc, ss * 512:(ss + 1) * 512],
                        ps[:], b_sb[:, dc, :], None, mybir.AluOpType.add,
                    )
            return [lambda i=i: piece(i) for i in range(4)]

        def kcol_piece(dc, kb, pool=None, tag=None):
            """One ~430ns piece: full-contraction k-proj for one 128-col
            block of kt, so sc(kb) consumers get per-kb granularity."""
            def piece():
                ps = (pool or psp).tile([P, P], F32,
                                        tag=tag or next_tag(), name="ps")
                for fc in range(FC):
                    nc.tensor.matmul(
                        ps[:],
                        wk_sb[:, fc, dc * P:(dc + 1) * P],
                        xk_sb[:, fc, kb * P:(kb + 1) * P],
                        start=(fc == 0), stop=(fc == FC - 1),
                    )
                nc.vector.tensor_scalar(
                    kt_sb[:, dc, kb * P:(kb + 1) * P],
                    ps[:], bk_sb[:, dc, :], None, mybir.AluOpType.add,
                )
            return [piece]

        def v_chain_piece(sb, hhalf):
            """One ~430ns piece: 8 matmuls of N=128 for 2 heads + evac."""
            tag = next_tag()

            def piece():
                ps = psp.tile([P, P], F32, tag=tag, name="ps")
                for fc in range(FC):
                    nc.tensor.matmul(
                        ps[:],
                        xv_sb[:, fc, sb * P:(sb + 1) * P],
                        wv_sb[:, fc, hhalf * P:(hhalf + 1) * P],
                        start=(fc == 0), stop=(fc == FC - 1),
                    )
                nc.vector.tensor_tensor(
                    v_sb[:, sb, 2 * hhalf:2 * hhalf + 2, :DK],
                    ps[:].rearrange("p (h d) -> p h d", h=2),
                    bv_rep[:, hhalf * P:(hhalf + 1) * P]
                        .rearrange("p (h d) -> p h d", h=2),
                    mybir.AluOpType.add,
                )
            return [piece]

        def bv_piece():
            def piece():
                ps = psp.tile([P, DC], F32, tag=next_tag(), name="ps")
                nc.tensor.matmul(ps[:], ones_f32[:], bv_row[:],
                                 start=True, stop=True)
                nc.vector.tensor_copy(bv_rep[:], ps[:])
            return [piece]

        def oproj_piece(pool, sb_i, split_evac=False):
            """One s-block of the O-projection as two ~430ns sub-pieces
            (one 512-col psum chain + evac each); the second also issues
            the merged [128, 1024] DMA. split_evac puts half 1's
            evacuation on the (tail-idle) ACT."""
            state = {}

            def half_piece(half):
                if half == 0:
                    state["o"] = opool.tile([P, D], BF16, tag="o", name="o_sb")
                o_sb = state["o"]
                ps = pool.tile([P, 512], F32, tag=next_tag(), name="ps")
                for pair in range(2):
                    nc.tensor.matmul(
                        ps[:],
                        outt_sb[:, pair, sb_i * P:(sb_i + 1) * P],
                        wo_sb[:, pair, half * 512:(half + 1) * 512],
                        start=(pair == 0), stop=(pair == 1),
                    )
                if split_evac and half == 1:
                    nc.scalar.activation(
                        o_sb[:, half * 512:(half + 1) * 512], ps[:],
                        mybir.ActivationFunctionType.Copy,
                        bias=0.0, scale=1.0,
                    )
                else:
                    nc.vector.tensor_copy(
                        o_sb[:, half * 512:(half + 1) * 512], ps[:])
                if half == 1:
                    nc.sync.dma_start(
                        outp[sb_i * P:(sb_i + 1) * P, :], o_sb[:])
            return [lambda h=h: half_piece(h) for h in range(2)]

        # ---- deadline-ordered work queue: (deadline, ready, piece) ----
        # drained one per step when the head's inputs exist (ready <= s),
        # plus everything past its deadline
        work = deque()

        def enq(deadline, pieces, ready=0):
            for p in pieces:
                work.append((deadline, ready, p))

        def drain(s):
            # Tile deps are trace-ordered: a piece MUST be emitted by its
            # deadline (one step before its consumer) no matter what;
            # ready only gates the opportunistic early pops.
            n = 0
            while work:
                dl, rdy, fn = work[0]
                if dl <= s + 1 or (n < 1 and rdy <= s):
                    work.popleft()
                    fn()
                    n += 1
                else:
                    break

        # phase A: only what sc/exp of (qs0, pair0, kb0..3) needs, in DMA
        # arrival order (xq lands first); the two later kcols borrow
        # still-unused sc-pool banks so phase A doesn't serialize on the
        # two projection-psum tags
        for fn in qk_chain_pieces(xq_sb, wq_sb, bq_sb, qt_sb, 0, 0,
                                  pool=sc_pool, tag="sc"):
            fn()
        kcol_piece(0, 0)[0]()
        kcol_piece(0, 1)[0]()
        kcol_piece(0, 2, pool=sc_pool, tag="sc")[0]()
        kcol_piece(0, 3)[0]()

        enq(2, bv_piece())
        for kb in range(4, KB):
            enq(max(0, kb - 2), kcol_piece(0, kb),
                ready=land_step("wk", f"xk{kb // 4}"))
        for sb in range(KB):
            enq(sb + LAG - 1, v_chain_piece(sb, 0),
                ready=land_step("wv", f"xv{sb // 4}"))
        for ss in range(1, SSN):
            enq(16 * ss - 4, qk_chain_pieces(xq_sb, wq_sb, bq_sb, qt_sb, 0, ss),
                ready=land_step("wq", f"xq{ss}"))
        for kb in range(KB):
            enq(60 + kb, kcol_piece(1, kb),
                ready=land_step("wk", f"xk{kb // 4}"))
        for ss in range(SSN):
            enq(60 + 16 * ss, qk_chain_pieces(xq_sb, wq_sb, bq_sb, qt_sb, 1, ss),
                ready=land_step("wq", f"xq{ss}"))
        for sb in range(KB):
            enq(64 + sb + LAG - 3, v_chain_piece(sb, 1),
                ready=land_step("wv", f"xv{sb // 4}"))
        # O-projection for qs 0/1/2 once both pairs' transposes are done
        # (norm of block 4+qs is emitted at step 16*(4+qs)+15+LAG)
        for i in range(4):
            for qs in range(3):
                rd = 16 * (4 + qs) + 19 + LAG
                enq(rd + 3 * i, oproj_piece(psp, 4 * qs + i), ready=rd)
        # sub-pieces of one sb must stay adjacent after the deadline sort
        # (they share an o_sb tile); deadlines above are unique per sb
        work = deque(sorted(work, key=lambda x: x[0]))

        # ---- the 128-step pipeline ----
        pt_tiles = {}
        av_tiles = {}

        def emit_sc_exp(s):
            qs, pair = blocks[s // KB]
            kb = s % KB
            q0 = qs * QSUP
            sc_ps = sc_pool.tile([P, 2, QSUP], F32, tag="sc", name="sc")
            for hh in range(2):
                hp = hh * DK
                nc.tensor.matmul(
                    sc_ps[:, hh, :],
                    kt_sb[hp:hp + DK, pair, kb * P:(kb + 1) * P],
                    qt_sb[hp:hp + DK, pair, q0:q0 + QSUP],
                    start=True, stop=True,
                )
            pt = ptpool.tile([P, 2, QSUP], BF16, tag="pt", name="pt")
            nc.scalar.activation(
                pt[:], sc_ps[:], mybir.ActivationFunctionType.Exp,
                bias=0.0, scale=0.125,
            )
            pt_tiles[s] = pt

        def emit_av(s):
            blk = s // KB
            qs, pair = blocks[blk]
            kb = s % KB
            pt = pt_tiles.pop(s)
            if kb == 0:
                av_tiles[blk] = [
                    av_pool.tile([P, NQ, DK + 1], F32, tag="av", name=f"av{hh}")
                    for hh in range(2)
                ]
            av = av_tiles[blk]
            for hh in range(2):
                h = 2 * pair + hh
                for qt in range(NQ):
                    # one start=True per PSUM bank: the bank-granular
                    # pending-zero covers the other interleaved chains
                    nc.tensor.matmul(
                        av[hh][:, qt, :],
                        pt[:, hh, qt * P:(qt + 1) * P],
                        v_sb[:, kb, h, :],
                        start=(kb == 0 and qt == 0),
                        stop=(kb == KB - 1 and qt == NQ - 1),
                        skip_group_check=True,
                    )

        def emit_norm_transpose(blk, skip_transpose=False):
            qs, pair = blocks[blk]
            av = av_tiles.pop(blk)
            att = attpool.tile([P, NQ, 2, DK], BF16, tag="att", name="att")
            for hh in range(2):
                rec = dyn.tile([P, NQ, 1], F32, tag=f"rec{hh}", name="rec")
                nc.vector.reciprocal(rec[:, :, 0], av[hh][:, :, DK])
                nc.vector.tensor_tensor(
                    att[:, :, hh, :],
                    av[hh][:, :, :DK],
                    rec[:].broadcast_to([P, NQ, DK]),
                    mybir.AluOpType.mult,
                )
            if skip_transpose:
                return att
            for qt in range(NQ):
                qg = qs * NQ + qt
                nc.sync.dma_start_transpose(
                    outt_sb[:, pair, qg * P:(qg + 1) * P],
                    att[:, qt, :, :],
                )

        for s in range(NSTEP):
            emit_sc_exp(s)
            if s - LAG >= 0:
                emit_av(s - LAG)
                if (s - LAG) % KB == KB - 1:
                    emit_norm_transpose((s - LAG) // KB)
            drain(s)
        att_last = None
        for s in range(NSTEP, NSTEP + LAG):
            emit_av(s - LAG)
            if (s - LAG) % KB == KB - 1:
                att_last = emit_norm_transpose((s - LAG) // KB,
                                               skip_transpose=True)
        while work:
            work.popleft()[2]()

        # ---- tail: last block's transposes on the (idle) PE via
        # is_transpose + identity, then qs-3 O-projection on the freed psum
        # banks; evacs alternate DVE / the now-idle ACT ----
        psp.release()
        av_pool.release()
        sc_pool.release()
        op2 = tc.alloc_tile_pool(name="op2", bufs=3, space="PSUM")
        qs_l, pair_l = blocks[NBLK - 1]
        for qt in range(NQ):
            tr_ps = op2.tile([P, P], BF16, tag="tr", bufs=2, name="tr")
            nc.tensor.matmul(tr_ps[:], att_last[:, qt, :, :], ident_sb[:],
                             is_transpose=True, start=True, stop=True)
            qg = qs_l * NQ + qt
            nc.vector.tensor_copy(
                outt_sb[:, pair_l, qg * P:(qg + 1) * P], tr_ps[:])
        for qt in range(NQ):
            for fn in oproj_piece(op2, 4 * qs_l + qt, split_evac=True):
                fn()

        for pool in (op2, opool, dyn, attpool, ptpool, qkv, consts):
            pool.release()

    split_multi_waits(nc)
    return nc


_NC_CACHE = None


def prep_in_maps(Q, K, V, W_q, b_q, W_k, b_k, W_v, b_v, W_o, b_o):
    """Host-side sharding: per-core input dicts (transposed, bf16-cast)."""
    bf = ml_dtypes.bfloat16
    Q, K, V = np.asarray(Q), np.asarray(K), np.asarray(V)
    xt = {}
    for b in range(B):
        xt[b] = (
            np.ascontiguousarray(Q[b].T).astype(bf),
            np.ascontiguousarray(K[b].T).astype(bf),
            np.ascontiguousarray(V[b].T).astype(bf),
        )
    in_maps = []
    for c in range(N_CORES):
        b = c // 4
        g = c % 4
        sl = slice(g * DC, (g + 1) * DC)
        in_maps.append({
            "xtq": xt[b][0], "xtk": xt[b][1], "xtv": xt[b][2],
            "wqt": np.ascontiguousarray(np.asarray(W_q)[sl, :].T).astype(bf),
            "wkt": np.ascontiguousarray(np.asarray(W_k)[sl, :].T).astype(bf),
            "wvt": np.ascontiguousarray(np.asarray(W_v)[sl, :].T).astype(bf),
            "wot": np.ascontiguousarray(np.asarray(W_o)[:, sl].T).astype(bf),
            "bq": np.asarray(b_q)[sl].reshape(DC, 1).astype(np.float32),
            "bk": np.asarray(b_k)[sl].reshape(DC, 1).astype(np.float32),
            "bvr": np.asarray(b_v)[sl].reshape(1, DC).astype(np.float32),
            "ident": np.eye(P, dtype=bf),
        })
    return in_maps


def gather_out(partials, b_o):
    """Host-side unshard: sum the four W_o-row partials per batch + b_o."""
    out = np.zeros((B, S, D), np.float32)
    for c in range(N_CORES):
        out[c // 4] += np.asarray(partials[c]).astype(np.float32)
    out += np.asarray(b_o).astype(np.float32)
    return out


def kernel(Q, K, V, W_q, b_q, W_k, b_k, W_v, b_v, W_o, b_o):
    global _NC_CACHE
    in_maps = prep_in_maps(Q, K, V, W_q, b_q, W_k, b_k, W_v, b_v, W_o, b_o)
    if _NC_CACHE is None:
        _NC_CACHE = build_bass()
    res = run_bass_kernel_spmd(_NC_CACHE, in_maps, core_ids=list(range(N_CORES)))
    return gather_out([res.results[c]["outp"] for c in range(N_CORES)], b_o)

